# revision 40
# baseline (speedup 1.0000x reference)
"""Trainium2 Bass kernel for the CustomLossFilter loss.

reference semantics (per row, fp32):
    cond = |inputs[:,4] - inputs[:,2]| < 0.1
    diff = where(cond, inputs[:,0] - inputs[:,4], inputs[:,0] - targets[:,0])
    out  = mean(|diff|)

Strategy: data-parallel over the 20M rows across 8 NeuronCores (2.5M rows
per core).  Inside a core, rows are mapped [128 partitions x 19531 rows]
with each partition owning a contiguous row range, so every DMA is a plain
contiguous 2D transfer.  Columns 0/2/4 are accessed with stride-5 APs in
SBUF.  The kernel is HBM-bound (60 MB/core at ~358 GB/s ~= 168 us), so the
compute chain is spread across GpSimd (cond subtract), Vector (mask +
select + diff) and Scalar (abs+accumulate) to keep every engine well under
the DMA budget, and the trailing tiles taper down so the pipeline drains
quickly after the last DMA.  Each core emits a [128,1] vector of
per-partition |diff| sums; the host adds the 1024 partials and divides by N.
"""

import numpy as np

import concourse.bacc as bacc
import concourse.mybir as mybir
from concourse import tile
from concourse.bass_utils import run_bass_kernel_spmd

N_TOTAL = 20_000_000
F = 5
N_CORES = 8
ROWS = N_TOTAL // N_CORES  # 2_500_000 rows per core
P = 128
W = 2048  # rows per partition per main tile
ERR_OK = 0.1

_ALU = mybir.AluOpType
_AX = mybir.AxisListType
_F32 = mybir.dt.float32
_U8 = mybir.dt.uint8
_ABS = mybir.ActivationFunctionType.Abs
_CPY = mybir.ActivationFunctionType.Copy


def _widths(rpp, w):
    """Main tiles of width w — the last one split in half so its compute
    finishes with the DMA stream — then a split remainder so the last tile
    is small and the post-DMA pipeline drain is short."""
    full, rem = divmod(rpp, w)
    widths = [w] * full
    c = 480
    if rem > (c * 3) // 2:
        widths.extend([rem - c, c])
    elif rem:
        widths.append(rem)
    return widths


def _body(tc, inp, tgt, out, rows, w):
    nc = tc.nc
    rpp = rows // P          # rows per partition in the main region
    scrap = rows - P * rpp   # leftover rows (< 128)

    widths = _widths(rpp, w)
    nt = len(widths) + (1 if scrap else 0)

    # [128, rpp*5] / [128, rpp] contiguous-per-partition views of DRAM
    in_main = inp[: P * rpp, :].rearrange("(p r) f -> p (r f)", p=P)
    tg_main = tgt[: P * rpp, :].rearrange("(p r) f -> p (r f)", p=P)

    with (
        tc.tile_pool(name="acc", bufs=1) as accpool,
        tc.tile_pool(name="inp", bufs=3) as inpool,
        tc.tile_pool(name="tgp", bufs=3) as tgpool,
        tc.tile_pool(name="dfp", bufs=2) as dpool,
        tc.tile_pool(name="abp", bufs=2) as apool,
        tc.tile_pool(name="msk", bufs=2) as mpool,
    ):
        acc = accpool.tile([P, nt], _F32)
        nc.gpsimd.memset(acc[:], 0.0)

        off = 0
        for t, wt in enumerate(widths):
            ti = inpool.tile([P, w * F], _F32, tag="in")
            tt = tgpool.tile([P, w], _F32, tag="tg")
            nc.sync.dma_start(ti[:, : wt * F], in_main[:, off * F : (off + wt) * F])
            nc.scalar.dma_start(tt[:, :wt], tg_main[:, off : off + wt])

            in0 = ti[:, 0 : wt * F : F]
            in2 = ti[:, 2 : wt * F : F]
            in4 = ti[:, 4 : wt * F : F]

            d = dpool.tile([P, w], _F32, tag="d")
            m = mpool.tile([P, w], _U8, tag="m")
            diff = dpool.tile([P, w], _F32, tag="f")
            adiff = apool.tile([P, w], _F32, tag="ab")  # write-only scratch
            du = d[:, :wt].bitcast(mybir.dt.uint32)
            lite = t >= len(widths) - 2
            if not lite:
                # Steady state: cond subtract on the (otherwise idle) GpSimd
                # engine and the col-0 deinterleave on the Scalar engine, so
                # Vector stays well under the DMA cadence and the Scalar
                # queue (which also carries the final abs+accum) never
                # serializes the pipeline.  |d| < 0.1f is computed exactly
                # in int space: clear the sign bit, then unsigned-compare
                # against the bit pattern of 0.1f (positive IEEE754 floats
                # order like ints).
                c0 = apool.tile([P, w], _F32, tag="ab")  # shares ring w/ adiff
                nc.gpsimd.tensor_tensor(d[:, :wt], in4, in2, _ALU.subtract)
                nc.scalar.activation(c0[:, :wt], in0, _CPY)
                nc.vector.tensor_scalar(
                    du, du, 0x7FFFFFFF, None, _ALU.bitwise_and
                )
                nc.vector.tensor_scalar(
                    m[:, :wt], du, 0x3DCCCCCD, None, _ALU.is_lt
                )
                nc.vector.copy_predicated(tt[:, :wt], m[:, :wt], in4)
                nc.vector.tensor_tensor(
                    diff[:, :wt], c0[:, :wt], tt[:, :wt], _ALU.subtract
                )
            else:
                # Tail tiles: the DMA stream is over, so latency (not
                # throughput) matters — run the chain on Vector to avoid
                # GpSimd dispatch overhead; the col-0 copy still runs on
                # Scalar in parallel with Vector's mask ops.
                c0 = apool.tile([P, w], _F32, tag="ab")
                nc.scalar.activation(c0[:, :wt], in0, _CPY)
                nc.vector.tensor_tensor(d[:, :wt], in4, in2, _ALU.subtract)
                nc.vector.tensor_scalar(
                    du, du, 0x7FFFFFFF, None, _ALU.bitwise_and
                )
                nc.vector.tensor_scalar(
                    m[:, :wt], du, 0x3DCCCCCD, None, _ALU.is_lt
                )
                nc.vector.copy_predicated(tt[:, :wt], m[:, :wt], in4)
                nc.vector.tensor_tensor(
                    diff[:, :wt], c0[:, :wt], tt[:, :wt], _ALU.subtract
                )
            nc.scalar.activation(
                adiff[:, :wt], diff[:, :wt], _ABS, accum_out=acc[:, t : t + 1]
            )
            off += wt

            if t == 0 and scrap:
                # tiny leftover block: emit early so it never sits in the
                # pipeline tail
                si = inpool.tile([scrap, F], _F32, tag="sin")
                st = tgpool.tile([scrap, 1], _F32, tag="stg")
                nc.sync.dma_start(si[:], inp[P * rpp :, :])
                nc.scalar.dma_start(st[:], tgt[P * rpp :, :])
                sd = dpool.tile([scrap, 1], _F32, tag="sd")
                sm = mpool.tile([scrap, 1], _U8, tag="sm")
                sa = apool.tile([scrap, 1], _F32, tag="sb")
                sdu = sd[:].bitcast(mybir.dt.uint32)
                nc.vector.tensor_tensor(sd[:], si[:, 4:5], si[:, 2:3], _ALU.subtract)
                nc.vector.tensor_scalar(sdu, sdu, 0x7FFFFFFF, None, _ALU.bitwise_and)
                nc.vector.tensor_scalar(sm[:], sdu, 0x3DCCCCCD, None, _ALU.is_lt)
                nc.vector.copy_predicated(st[:], sm[:], si[:, 4:5])
                sdiff = dpool.tile([scrap, 1], _F32, tag="sd")
                nc.vector.tensor_tensor(sdiff[:], si[:, 0:1], st[:], _ALU.subtract)
                nc.scalar.activation(
                    sa[:], sdiff[:], _ABS, accum_out=acc[:scrap, nt - 1 : nt]
                )

        # ship the raw per-tile column sums; the host adds the 8*128*nt
        # partials (cheaper than an on-chip reduce at the very end of the
        # pipeline drain)
        nc.sync.dma_start(out[:], acc[:])


def n_tiles(rows=ROWS, w=W):
    rpp = rows // P
    scrap = rows - P * rpp
    return len(_widths(rpp, w)) + (1 if scrap else 0)


def build_nc(rows=ROWS, w=W):
    nc = bacc.Bacc(
        "TRN2", target_bir_lowering=False, debug=False, num_devices=N_CORES
    )
    inp = nc.dram_tensor("inputs", [rows, F], _F32, kind="ExternalInput").ap()
    tgt = nc.dram_tensor("targets", [rows, 1], _F32, kind="ExternalInput").ap()
    out = nc.dram_tensor(
        "out", [P, n_tiles(rows, w)], _F32, kind="ExternalOutput"
    ).ap()
    with tile.TileContext(nc) as tc:
        _body(tc, inp, tgt, out, rows, w)
    nc.compile()
    return nc


_NC_CACHE = {}


def _get_nc():
    if "nc" not in _NC_CACHE:
        _NC_CACHE["nc"] = build_nc()
    return _NC_CACHE["nc"]


def run_sharded(inputs, targets, **spmd_kwargs):
    """Run the SPMD kernel; returns (per-core [128,1] partials, results obj)."""
    nc = _get_nc()
    inputs = np.asarray(inputs, dtype=np.float32)
    targets = np.asarray(targets, dtype=np.float32)
    in_maps = [
        {
            "inputs": inputs[i * ROWS : (i + 1) * ROWS],
            "targets": targets[i * ROWS : (i + 1) * ROWS],
        }
        for i in range(N_CORES)
    ]
    res = run_bass_kernel_spmd(nc, in_maps, list(range(N_CORES)), **spmd_kwargs)
    partials = np.stack([r["out"] for r in res.results])  # [8, 128, n_tiles]
    return partials, res


def kernel(inputs, targets):
    partials, _ = run_sharded(inputs, targets)
    total = partials.astype(np.float64).sum()
    return np.asarray(total / N_TOTAL, dtype=np.float32)
